# revision 43
# baseline (speedup 1.0000x reference)
"""Cosine multihead attention on 8 Trainium2 NeuronCores.

Sharding: batch*heads across cores. Core c handles batch b = c // 4 and the
4 heads [4*(c%4), 4*(c%4)+4). Each core computes its heads' q/k/v projections
(tensor-parallel slices of in_proj), full attention for its (B,H) slice, and a
partial bf16 out-projection (rank-256 contribution). The host sums the 4
partials per batch and adds out_proj_bias.

Schedule: steady state is ACT(exp)-bound (one [128,1024] Exp per (qb,kc)
attention iteration, ~1.15us), with ~0.5us/iter of PE slack. Everything else
is explicitly interleaved into that slack via a filler deque: after each
attention iteration we pop filler ops (v projection, pair-1 q/k projections,
norms, out-projection units) up to a per-iteration PE-time budget. Emission
order = Tile priority, so the static schedule interleaves exactly as emitted.

Norms use one ACT table set (natural_log_exp_and_others, manually preloaded +
drained once): 1/(||x||*tau) = Exp(-0.5*Ln(sumsq) - ln tau). No sqrt set, so
no table thrash against the attention Exp stream. Norm broadcast matmuls are
bf16 (216ns vs 1055ns fp32). Softmax denominators come free from a
ones-column in v (M=65 PV); 1/z is a DVE fast-recip read straight from PSUM,
broadcast with a bf16 matmul.

Device layout:
- q,k projected transposed (head_dim on partitions, seq on free) so QK^T
  needs no on-chip transpose; v natural so it is the PV stationary operand.
- QK^T runs 2 heads concurrently via PE row tiling (K=64 at bases 0 and 64).
- All bf16 matmuls with fp32 PSUM accumulation; softmax math in fp32.
"""

import sys

if "/opt/trn_rl_repo" not in sys.path:
    sys.path.insert(0, "/opt/trn_rl_repo")

import numpy as np
import ml_dtypes

import concourse.bass as bass
import concourse.tile as tile
from concourse import bacc, mybir

S, B, E, H = 2048, 2, 1024, 16
HD = E // H            # 64
HPC = 4                # heads per core
NCORES = 8
TAU_MIN = 0.01

BF16 = ml_dtypes.bfloat16
DT_BF = mybir.dt.bfloat16
DT_F32 = mybir.dt.float32

KC_E = E // 128        # 8 contraction chunks for projections
MQ = S // 128          # 16 seq chunks of 128
NPAIR = HPC // 2       # 2 head pairs per core

ACT_SET_LN_EXP = 6     # natural_log_exp_and_others in act_info.json
ITER_FILL_NS = 560     # PE-time filler budget per attention iteration
PRIO_ATTN = 1_000_000  # priority boost: attention preempts ready fillers


def build_program():
    nc = bacc.Bacc(None)

    xq = nc.dram_tensor("xq_t", [E, S], DT_BF, kind="ExternalInput")
    xk = nc.dram_tensor("xk_t", [E, S], DT_BF, kind="ExternalInput")
    xv = nc.dram_tensor("xv_t", [E, S], DT_BF, kind="ExternalInput")
    wq = nc.dram_tensor("wq_t", [E, 256], DT_BF, kind="ExternalInput")
    wk = nc.dram_tensor("wk_t", [E, 256], DT_BF, kind="ExternalInput")
    wv = nc.dram_tensor("wv_t", [E, 256], DT_BF, kind="ExternalInput")
    bq = nc.dram_tensor("b_q", [1, 256], DT_BF, kind="ExternalInput")
    bk = nc.dram_tensor("b_k", [1, 256], DT_BF, kind="ExternalInput")
    bv = nc.dram_tensor("b_v", [1, 256], DT_BF, kind="ExternalInput")
    wo = nc.dram_tensor("wo_t", [256, E], DT_BF, kind="ExternalInput")
    nbias_in = nc.dram_tensor("nbias", [2, 2], DT_F32, kind="ExternalInput")
    sel2_in = nc.dram_tensor("sel2", [2, 128], DT_BF, kind="ExternalInput")
    outp = nc.dram_tensor("out_p", [S, E], DT_BF, kind="ExternalOutput")

    with tile.TileContext(nc) as tc:
        with (
            tc.tile_pool(name="consts", bufs=1) as consts,
            tc.tile_pool(name="xin", bufs=1) as xin,
            tc.tile_pool(name="wts", bufs=1) as wts,
            tc.tile_pool(name="qk", bufs=1) as qkpool,
            tc.tile_pool(name="vsb", bufs=1) as vpool,
            tc.tile_pool(name="work", bufs=3) as work,
            tc.tile_pool(name="wk2", bufs=2) as wk2,
            tc.tile_pool(name="sqp", bufs=2) as sqp,
            tc.tile_pool(name="outs", bufs=3) as outs,
            tc.tile_pool(name="ps_sc", bufs=2, space="PSUM") as ps_sc,
            tc.tile_pool(name="ps_acc", bufs=2, space="PSUM") as ps_acc,
            tc.tile_pool(name="ps_fill", bufs=2, space="PSUM") as ps_fill,
        ):
            # ---- ACT table preload: combined ln+exp set, loaded once.
            # The DRAIN is required: the table DMA is async and the first
            # ACTIVATE would race it on the first execution.
            nc.scalar.add_instruction(
                mybir.InstLoadActFuncSet(
                    name=nc.get_next_instruction_name(),
                    act_func_set_id=ACT_SET_LN_EXP,
                    ins=[],
                    outs=[],
                )
            )
            drain = mybir.InstDrain(
                name=nc.get_next_instruction_name(),
                ins=[],
                outs=[],
                bass_is_fusable=False,
            )
            drain.engine = nc.scalar.engine
            nc.scalar.add_instruction(drain)

            # ---- constants -------------------------------------------------
            ones_row = consts.tile([1, 512], DT_BF, tag="ones_row")
            nc.vector.memset(ones_row, 1.0)
            ones_hi = consts.tile([128, 64], DT_BF, tag="ones_hi")
            nc.vector.memset(ones_hi, 1.0)
            hsel = consts.tile([128, 2], DT_BF, tag="hsel")
            nc.vector.memset(hsel, 0.0)
            nc.vector.memset(hsel[0:64, 0:1], 1.0)
            nc.vector.memset(hsel[64:128, 1:2], 1.0)
            # table-warm dummy (nothing consumes it)
            warm = consts.tile([1, 64], DT_F32, tag="warm")
            nc.vector.memset(warm, 1.0)
            nc.scalar.activation(warm, warm, mybir.ActivationFunctionType.Exp)

            sel2 = consts.tile([2, 128], DT_BF, tag="sel2")
            nbias_sb = consts.tile([2, 2], DT_F32, tag="nbias")
            bq_sb = consts.tile([1, 256], DT_BF, tag="bq")
            bk_sb = consts.tile([1, 256], DT_BF, tag="bk")
            bv_sb = consts.tile([1, 256], DT_BF, tag="bv")

            # ---- DMA plan --------------------------------------------------
            # gpsimd(SWDGE): all weights/consts (~1.6MB, done early)
            # sync(HWDGE):   xk, then xv chunks 0-3
            # scalar(HWDGE): xq, then xv chunks 4-7 (ACT idle during lead-in)
            # Per-chunk tiles everywhere: SBUF RAW dependencies are
            # tile-granular, so consumers become ready per chunk instead of
            # waiting for whole tensors.
            wq_sb = [wts.tile([128, 256], DT_BF, tag=f"wq{c}", name=f"wq{c}")
                     for c in range(KC_E)]
            wk_sb = [wts.tile([128, 256], DT_BF, tag=f"wk{c}", name=f"wk{c}")
                     for c in range(KC_E)]
            wv_sb = [wts.tile([128, 256], DT_BF, tag=f"wv{c}", name=f"wv{c}")
                     for c in range(KC_E)]
            xq_sb = [xin.tile([128, S], DT_BF, tag=f"xq{c}", name=f"xq{c}")
                     for c in range(KC_E)]
            xk_sb = [xin.tile([128, S], DT_BF, tag=f"xk{c}", name=f"xk{c}")
                     for c in range(KC_E)]
            xv_sb = [xin.tile([128, S], DT_BF, tag=f"xv{c}", name=f"xv{c}")
                     for c in range(KC_E)]
            wo_sb = [wts.tile([128, E], DT_BF, tag=f"wo{c}", name=f"wo{c}")
                     for c in range(2)]

            for c in range(KC_E):
                nc.gpsimd.dma_start(out=wk_sb[c], in_=wk[c * 128:(c + 1) * 128, :])
            for c in range(KC_E):
                nc.gpsimd.dma_start(out=wq_sb[c], in_=wq[c * 128:(c + 1) * 128, :])
            nc.gpsimd.dma_start(out=bk_sb, in_=bk[:, :])
            nc.gpsimd.dma_start(out=bq_sb, in_=bq[:, :])
            nc.gpsimd.dma_start(out=nbias_sb, in_=nbias_in[:, :])
            nc.gpsimd.dma_start(out=sel2, in_=sel2_in[:, :])
            for c in range(KC_E):
                nc.sync.dma_start(out=xk_sb[c], in_=xk[c * 128:(c + 1) * 128, :])
                nc.scalar.dma_start(out=xq_sb[c], in_=xq[c * 128:(c + 1) * 128, :])
            for c in range(KC_E):
                nc.gpsimd.dma_start(out=wv_sb[c], in_=wv[c * 128:(c + 1) * 128, :])
            nc.gpsimd.dma_start(out=bv_sb, in_=bv[:, :])
            for c in range(KC_E):
                nc.gpsimd.dma_start(out=xv_sb[c], in_=xv[c * 128:(c + 1) * 128, :])
            for c in range(2):
                nc.gpsimd.dma_start(out=wo_sb[c], in_=wo[c * 128:(c + 1) * 128, :])

            # q/k/heads as per-512-block tiles (4 blocks per pair)
            qt = [[qkpool.tile([128, 512], DT_BF, tag=f"qt{p}_{b}",
                               name=f"qt{p}_{b}") for b in range(4)]
                  for p in range(NPAIR)]
            kt = [[qkpool.tile([128, 512], DT_BF, tag=f"kt{p}_{b}",
                               name=f"kt{p}_{b}") for b in range(4)]
                  for p in range(NPAIR)]
            heads_t = [[qkpool.tile([128, 512], DT_BF, tag=f"ht{p}_{b}",
                                    name=f"ht{p}_{b}") for b in range(4)]
                       for p in range(NPAIR)]
            # One tile per seq chunk: keeps the PV-read -> v-drain-write
            # dependency trackable at tile granularity (a single 4-D tile
            # with strided rearranged writes loses the RAW edge -> race).
            v_sb = [vpool.tile([128, HPC, HD + 1], DT_BF, tag=f"v{m}",
                               name=f"v{m}") for m in range(MQ)]

            def proj_norm_unit_ops(dst, w_sb, b_sb, x_sb, mc, n4, pool, ptag,
                                   with_tau, marker=None):
                """Op closures for one 512-col projection unit + its norm:
                8 accum matmuls + bias matmul -> drain -> square -> sumsq
                matmul -> Ln -> Exp (tau folded into bias) -> bf16 broadcast
                matmul -> in-place normalize. Returns [(pe_ns, fn), ...]."""
                sl = slice(n4 * 512, (n4 + 1) * 512)
                box = {}
                ops = []

                def mk_mm(c):
                    def go():
                        if c == 0:
                            box["pp"] = pool.tile([128, 512], DT_F32,
                                                  tag=ptag, name="pp_t")
                        nc.tensor.matmul(
                            box["pp"],
                            lhsT=w_sb[c][:, mc * 128:(mc + 1) * 128],
                            rhs=x_sb[c][:, sl],
                            start=(c == 0),
                            stop=False,
                        )
                    return go

                for c in range(KC_E):
                    ops.append((220, mk_mm(c)))

                def bias_mm():
                    nc.tensor.matmul(
                        box["pp"],
                        lhsT=b_sb[0:1, mc * 128:(mc + 1) * 128],
                        rhs=ones_row[0:1, 0:512],
                        start=False,
                        stop=True,
                    )
                ops.append((220, bias_mm))

                def drain_sq():
                    nc.vector.tensor_copy(out=dst, in_=box["pp"])
                    sq = sqp.tile([128, 512], DT_BF, tag="sq", name="sq_t")
                    nc.vector.tensor_mul(sq, dst, dst)
                    box["sq"] = sq
                ops.append((0, drain_sq))

                def sumsq():
                    ss = ps_fill.tile([2, 512], DT_F32, tag="fill", name="ss_t")
                    nc.tensor.matmul(ss, lhsT=hsel, rhs=box["sq"],
                                     start=True, stop=True)
                    box["ss"] = ss
                ops.append((220, sumsq))

                def ln_exp():
                    lt2 = wk2.tile([2, 512], DT_F32, tag="lt2", name="lt2_t")
                    nc.scalar.activation(lt2, box["ss"],
                                         mybir.ActivationFunctionType.Ln)
                    rr2 = wk2.tile([2, 512], DT_BF, tag="rr2", name="rr2_t")
                    bias = nbias_sb[:, mc:mc + 1] if with_tau else 0.0
                    nc.scalar.activation(rr2, lt2,
                                         mybir.ActivationFunctionType.Exp,
                                         bias=bias, scale=-0.5)
                    box["rr2"] = rr2
                ops.append((0, ln_exp))

                def bcast():
                    rb = ps_fill.tile([128, 512], DT_F32, tag="fill",
                                      name="rb_t")
                    nc.tensor.matmul(rb, lhsT=sel2, rhs=box["rr2"],
                                     start=True, stop=True)
                    box["rb"] = rb
                ops.append((220, bcast))

                def apply():
                    nc.vector.tensor_mul(dst, dst, box["rb"])
                if marker is None:
                    ops.append((0, apply))
                else:
                    ops.append((0, apply, marker))
                return ops

            def v_unit_ops(m):
                """v projection for seq chunk m, directly in interleaved
                (h, d+ones) PSUM layout so the drain is a plain full-tile
                copy (strided/rearranged SBUF writes lose RAW tracking)."""
                box = {}
                ops = []

                def mk_mm(c):
                    def go():
                        if c == 0:
                            box["vp"] = ps_fill.tile([128, 256], DT_F32,
                                                     tag="fill", name="vp_t")
                        nc.tensor.matmul(
                            box["vp"],
                            lhsT=xv_sb[c][:, m * 128:(m + 1) * 128],
                            rhs=wv_sb[c],
                            start=(c == 0),
                            stop=False,
                        )
                    return go

                for c in range(KC_E):
                    ops.append((115, mk_mm(c)))

                def bias_mm():
                    nc.tensor.matmul(
                        box["vp"],
                        lhsT=ones_row[0:1, 0:128],
                        rhs=bv_sb[0:1, :],
                        start=False,
                        stop=True,
                    )
                ops.append((115, bias_mm))

                def drain():
                    for h in range(HPC):
                        nc.vector.tensor_copy(
                            out=v_sb[m][:, h, 0:HD],
                            in_=box["vp"][:, h * HD:(h + 1) * HD],
                        )
                        nc.vector.memset(v_sb[m][:, h, HD:HD + 1], 1.0)
                ops.append((0, drain, ("v", m)))
                return ops

            def outproj_unit_ops(m, n2):
                sl_n = slice(n2 * 512, (n2 + 1) * 512)
                box = {}
                ops = []

                def mk_mm(c):
                    def go():
                        if c == 0:
                            box["op"] = ps_fill.tile([128, 512], DT_F32,
                                                     tag="fill", name="op_t")
                        nc.tensor.matmul(
                            box["op"],
                            lhsT=heads_t[c][m // 4][:, (m % 4) * 128:(m % 4 + 1) * 128],
                            rhs=wo_sb[c][:, sl_n],
                            start=(c == 0), stop=(c == 1),
                        )
                    return go

                ops.append((220, mk_mm(0)))
                ops.append((220, mk_mm(1)))

                def drain():
                    ob = outs.tile([128, 512], DT_BF, tag="ob", name="ob_t")
                    nc.vector.tensor_copy(ob, box["op"])
                    nc.sync.dma_start(
                        out=outp[m * 128:(m + 1) * 128, sl_n], in_=ob
                    )
                ops.append((0, drain))
                return ops

            def attention_pair(p):
                """Attention for head pair p, emitted with a large priority
                boost: the scheduler runs these ops as soon as their inputs
                are ready, preempting equal-ready projection/outproj work
                emitted at normal priority. Per-qb softmax drain is deferred
                into the next qb's iteration 2 so the exp stream flows
                across boundaries."""
                def mk_drain(qb, o0, o1):
                    sl_q = slice(qb * 512, (qb + 1) * 512)

                    def emit_drain():
                      with tc.high_priority(offset=PRIO_ATTN):
                        for hl, o in ((0, o0), (1, o1)):
                            zcb = work.tile([128, 512], DT_BF, tag="zcb",
                                            name="zcb_t")
                            nc.vector.tensor_copy(zcb[64:65, :], o[64:65, :])
                            zb = ps_sc.tile([64, 512], DT_F32, tag="sc",
                                            name="zb_t")
                            nc.tensor.matmul(
                                zb,
                                lhsT=ones_hi[64:65, 0:64],
                                rhs=zcb[64:65, :],
                                start=True, stop=True,
                            )
                            zbi = work.tile([64, 512], DT_F32, tag="zbi",
                                            name="zbi_t")
                            nc.vector.reciprocal_approx_fast(out=zbi, in_=zb)
                            if hl == 0:
                                nc.vector.tensor_mul(
                                    heads_t[p][qb][0:64, :], o[0:64, :], zbi
                                )
                            else:
                                t2 = work.tile([64, 512], DT_BF, tag="t2",
                                               name="t2_t")
                                nc.vector.tensor_mul(t2, o[0:64, :], zbi)
                                nc.sync.dma_start(
                                    out=heads_t[p][qb][64:128, :], in_=t2
                                )
                    return emit_drain

                pending_drain = None
                for qb in range(4):
                    sl_q = slice(qb * 512, (qb + 1) * 512)
                    o0 = ps_acc.tile([128, 512], DT_F32, tag="oacc", name="o0_t")
                    o1 = ps_acc.tile([128, 512], DT_F32, tag="oacc", name="o1_t")
                    for kc in range(MQ):
                        with tc.high_priority(offset=PRIO_ATTN):
                            sc = ps_sc.tile([128, 1024], DT_F32, tag="sc", name="sc_t")
                            ktb = kt[p][kc // 4]
                            kcs = slice((kc % 4) * 128, (kc % 4 + 1) * 128)
                            nc.tensor.matmul(
                                sc[:, 0:512],
                                lhsT=ktb[0:64, kcs],
                                rhs=qt[p][qb][0:64, :],
                                start=True, stop=True,
                            )
                            nc.tensor.matmul(
                                sc[:, 512:1024],
                                lhsT=ktb[64:128, kcs],
                                rhs=qt[p][qb][64:128, :],
                                start=True, stop=True,
                            )
                            ex = work.tile([128, 1024], DT_BF, tag="exp", name="ex_t")
                            nc.scalar.activation(
                                ex, sc, mybir.ActivationFunctionType.Exp
                            )
                        if kc == 2 and pending_drain is not None:
                            pending_drain()
                            pending_drain = None
                        with tc.high_priority(offset=PRIO_ATTN):
                            nc.tensor.matmul(
                                o0[0:65, :],
                                lhsT=v_sb[kc][:, 2 * p, :],
                                rhs=ex[:, 0:512],
                                start=(kc == 0), stop=(kc == MQ - 1),
                            )
                            nc.tensor.matmul(
                                o1[0:65, :],
                                lhsT=v_sb[kc][:, 2 * p + 1, :],
                                rhs=ex[:, 512:1024],
                                start=(kc == 0), stop=(kc == MQ - 1),
                            )
                    pending_drain = mk_drain(qb, o0, o1)
                pending_drain()

            # ---- projections (normal priority; attention preempts) --------
            for n4 in range(4):
                for item in proj_norm_unit_ops(kt[0][n4], wk_sb, bk_sb, xk_sb,
                                               0, n4, ps_fill, "fill", True):
                    item[1]()
            for n4 in range(4):
                for item in proj_norm_unit_ops(qt[0][n4], wq_sb, bq_sb, xq_sb,
                                               0, n4, ps_fill, "fill", False):
                    item[1]()
            for m in range(MQ):
                for item in v_unit_ops(m):
                    item[1]()
            for n4 in range(4):
                for item in proj_norm_unit_ops(kt[1][n4], wk_sb, bk_sb, xk_sb,
                                               1, n4, ps_fill, "fill", True):
                    item[1]()
            for n4 in range(4):
                for item in proj_norm_unit_ops(qt[1][n4], wq_sb, bq_sb, xq_sb,
                                               1, n4, ps_fill, "fill", False):
                    item[1]()

            # ---- attention (priority-boosted) ------------------------------
            attention_pair(0)
            attention_pair(1)

            # ---- out-projection (normal priority; fills pair-1 gaps) -------
            for m in range(MQ):
                for n2 in range(2):
                    for item in outproj_unit_ops(m, n2):
                        item[1]()

    nc.compile()
    return nc


_CACHE = {}


def _get_program():
    if "nc" not in _CACHE:
        _CACHE["nc"] = build_program()
    return _CACHE["nc"]


def make_in_maps(query, key, value, in_proj_weight, in_proj_bias,
                 out_proj_weight, out_proj_bias, tau):
    query = np.asarray(query, np.float32)
    key = np.asarray(key, np.float32)
    value = np.asarray(value, np.float32)
    W = np.asarray(in_proj_weight, np.float32)
    bias = np.asarray(in_proj_bias, np.float32)
    Wo = np.asarray(out_proj_weight, np.float32)
    tau_c = np.maximum(np.asarray(tau, np.float32).reshape(H), TAU_MIN)

    # Transposed activations per batch: (E, S) bf16
    xT = {}
    for b in range(B):
        xT["q", b] = np.ascontiguousarray(query[:, b, :].T).astype(BF16)
        xT["k", b] = np.ascontiguousarray(key[:, b, :].T).astype(BF16)
        xT["v", b] = np.ascontiguousarray(value[:, b, :].T).astype(BF16)

    sel2_host = np.zeros((2, 128), np.float32)
    sel2_host[0, 0:64] = 1.0
    sel2_host[1, 64:128] = 1.0
    sel2_host = sel2_host.astype(BF16)

    in_maps = []
    for c in range(NCORES):
        b = c // 4
        h0 = HPC * (c % 4)
        rows = slice(h0 * HD, (h0 + HPC) * HD)
        rows_k = slice(E + h0 * HD, E + (h0 + HPC) * HD)
        rows_v = slice(2 * E + h0 * HD, 2 * E + (h0 + HPC) * HD)
        # per-pair -ln(tau) fold for the k-side norms: [head_lr, pair]
        nbias = np.zeros((2, 2), np.float32)
        for p in range(NPAIR):
            nbias[0, p] = -np.log(tau_c[h0 + 2 * p])
            nbias[1, p] = -np.log(tau_c[h0 + 2 * p + 1])
        in_maps.append({
            "xq_t": xT["q", b],
            "xk_t": xT["k", b],
            "xv_t": xT["v", b],
            "wq_t": np.ascontiguousarray(W[rows, :].T).astype(BF16),
            "wk_t": np.ascontiguousarray(W[rows_k, :].T).astype(BF16),
            "wv_t": np.ascontiguousarray(W[rows_v, :].T).astype(BF16),
            "b_q": bias[rows].reshape(1, 256).astype(BF16),
            "b_k": bias[rows_k].reshape(1, 256).astype(BF16),
            "b_v": bias[rows_v].reshape(1, 256).astype(BF16),
            "wo_t": np.ascontiguousarray(Wo[:, rows].T).astype(BF16),
            "nbias": nbias,
            "sel2": sel2_host,
        })
    return in_maps


def assemble_out(results, out_proj_bias):
    bo = np.asarray(out_proj_bias, np.float32)
    out = np.zeros((S, B, E), np.float32)
    for c in range(NCORES):
        out[:, c // 4, :] += np.asarray(results[c]["out_p"], np.float32)
    out += bo[None, None, :]
    return out


def kernel(query, key, value, in_proj_weight, in_proj_bias,
           out_proj_weight, out_proj_bias, tau):
    from concourse.bass_utils import run_bass_kernel_spmd
    nc = _get_program()
    in_maps = make_in_maps(query, key, value, in_proj_weight, in_proj_bias,
                           out_proj_weight, out_proj_bias, tau)
    res = run_bass_kernel_spmd(nc, in_maps, core_ids=list(range(NCORES)))
    return assemble_out(res.results, out_proj_bias)


if __name__ == "__main__":
    import reference

    inputs = {k: np.asarray(v) for k, v in reference.setup_inputs().items()}
    out = kernel(**inputs)
    print("out shape", out.shape, out.dtype)


# revision 44
# speedup vs baseline: 1.2632x; 1.2632x over previous
"""Cosine multihead attention on 8 Trainium2 NeuronCores.

Sharding: batch*heads across cores. Core c handles batch b = c // 4 and the
4 heads [4*(c%4), 4*(c%4)+4). Each core computes its heads' q/k/v projections
(tensor-parallel slices of in_proj), full attention for its (B,H) slice, and a
partial bf16 out-projection (rank-256 contribution). The host sums the 4
partials per batch and adds out_proj_bias.

Schedule: steady state is ACT(exp)-bound (one [128,1024] Exp per (qb,kc)
attention iteration, ~1.15us), with ~0.5us/iter of PE slack. v projection,
pair-1 q/k projections+norms and the out-projection are interleaved into that
slack via a filler deque popped between attention iterations. Tile creates
dependencies in EMISSION order, so any filler writing data that attention
reads must be emitted before the reading op: v units carry markers that
ensure_emitted() forces out before the PV that consumes them.

Norms use one ACT table set (natural_log_exp_and_others, manually preloaded +
drained once): 1/(||x||*tau) = Exp(-0.5*Ln(sumsq) - ln tau). No sqrt set, so
no table thrash against the attention Exp stream; norm broadcasts are bf16
matmuls. Softmax denominators come free from a ones-column in v (M=65 PV);
per-qb drains are deferred into the next qb's iteration 2 so the exp stream
flows across boundaries.

Device layout:
- q,k projected transposed (head_dim on partitions, seq on free) so QK^T
  needs no on-chip transpose; v natural so it is the PV stationary operand.
- QK^T runs 2 heads concurrently via PE row tiling (K=64 at bases 0 and 64).
- All bf16 matmuls with fp32 PSUM accumulation; softmax math in fp32.
"""

import sys
from collections import deque

if "/opt/trn_rl_repo" not in sys.path:
    sys.path.insert(0, "/opt/trn_rl_repo")

import numpy as np
import ml_dtypes

import concourse.bass as bass
import concourse.tile as tile
from concourse import bacc, mybir

S, B, E, H = 2048, 2, 1024, 16
HD = E // H            # 64
HPC = 4                # heads per core
NCORES = 8
TAU_MIN = 0.01

BF16 = ml_dtypes.bfloat16
DT_BF = mybir.dt.bfloat16
DT_F32 = mybir.dt.float32

KC_E = E // 128        # 8 contraction chunks for projections
MQ = S // 128          # 16 seq chunks of 128
NPAIR = HPC // 2       # 2 head pairs per core

ACT_SET_LN_EXP = 6     # natural_log_exp_and_others in act_info.json
ITER_FILL_NS = 560     # PE-time filler budget per attention iteration


def build_program():
    nc = bacc.Bacc(None)

    xq = nc.dram_tensor("xq_t", [E, S], DT_BF, kind="ExternalInput")
    xk = nc.dram_tensor("xk_t", [E, S], DT_BF, kind="ExternalInput")
    xv = nc.dram_tensor("xv_t", [E, S], DT_BF, kind="ExternalInput")
    wq = nc.dram_tensor("wq_t", [E, 256], DT_BF, kind="ExternalInput")
    wk = nc.dram_tensor("wk_t", [E, 256], DT_BF, kind="ExternalInput")
    wv = nc.dram_tensor("wv_t", [E, 256], DT_BF, kind="ExternalInput")
    bq = nc.dram_tensor("b_q", [1, 256], DT_BF, kind="ExternalInput")
    bk = nc.dram_tensor("b_k", [1, 256], DT_BF, kind="ExternalInput")
    bv = nc.dram_tensor("b_v", [1, 256], DT_BF, kind="ExternalInput")
    wo = nc.dram_tensor("wo_t", [256, E], DT_BF, kind="ExternalInput")
    nbias_in = nc.dram_tensor("nbias", [2, 2], DT_F32, kind="ExternalInput")
    sel2_in = nc.dram_tensor("sel2", [2, 128], DT_BF, kind="ExternalInput")
    outp = nc.dram_tensor("out_p", [S, E], DT_BF, kind="ExternalOutput")

    with tile.TileContext(nc) as tc:
        with (
            tc.tile_pool(name="consts", bufs=1) as consts,
            tc.tile_pool(name="xin", bufs=1) as xin,
            tc.tile_pool(name="wts", bufs=1) as wts,
            tc.tile_pool(name="qk", bufs=1) as qkpool,
            tc.tile_pool(name="vsb", bufs=1) as vpool,
            tc.tile_pool(name="work", bufs=3) as work,
            tc.tile_pool(name="wk2", bufs=2) as wk2,
            tc.tile_pool(name="sqp", bufs=2) as sqp,
            tc.tile_pool(name="outs", bufs=3) as outs,
            tc.tile_pool(name="ps_sc", bufs=2, space="PSUM") as ps_sc,
            tc.tile_pool(name="ps_acc", bufs=2, space="PSUM") as ps_acc,
            tc.tile_pool(name="ps_aux", bufs=1, space="PSUM") as ps_aux,
            tc.tile_pool(name="ps_fill", bufs=1, space="PSUM") as ps_fill,
        ):
            # ---- ACT table preload: combined ln+exp set, loaded once.
            # The DRAIN is required: the table DMA is async and the first
            # ACTIVATE would race it on the first execution.
            nc.scalar.add_instruction(
                mybir.InstLoadActFuncSet(
                    name=nc.get_next_instruction_name(),
                    act_func_set_id=ACT_SET_LN_EXP,
                    ins=[],
                    outs=[],
                )
            )
            drain = mybir.InstDrain(
                name=nc.get_next_instruction_name(),
                ins=[],
                outs=[],
                bass_is_fusable=False,
            )
            drain.engine = nc.scalar.engine
            nc.scalar.add_instruction(drain)

            # ---- constants -------------------------------------------------
            ones_row = consts.tile([1, 512], DT_BF, tag="ones_row")
            nc.vector.memset(ones_row, 1.0)
            ones_hi = consts.tile([128, 64], DT_BF, tag="ones_hi")
            nc.vector.memset(ones_hi, 1.0)
            hsel = consts.tile([128, 2], DT_BF, tag="hsel")
            nc.vector.memset(hsel, 0.0)
            nc.vector.memset(hsel[0:64, 0:1], 1.0)
            nc.vector.memset(hsel[64:128, 1:2], 1.0)
            # table-warm dummy (nothing consumes it)
            warm = consts.tile([1, 64], DT_F32, tag="warm")
            nc.vector.memset(warm, 1.0)
            nc.scalar.activation(warm, warm, mybir.ActivationFunctionType.Exp)

            sel2 = consts.tile([2, 128], DT_BF, tag="sel2")
            nbias_sb = consts.tile([2, 2], DT_F32, tag="nbias")
            bq_sb = consts.tile([1, 256], DT_BF, tag="bq")
            bk_sb = consts.tile([1, 256], DT_BF, tag="bk")
            bv_sb = consts.tile([1, 256], DT_BF, tag="bv")

            # ---- DMA plan --------------------------------------------------
            # gpsimd(SWDGE): weights/consts then xv then wo
            # sync(HWDGE):   xk and xq (k first per chunk)
            # Per-chunk input tiles: Tile RAW deps are tile-granular, so the
            # projections become ready chunk by chunk as DMAs land.
            wq_sb = [wts.tile([128, 256], DT_BF, tag=f"wq{c}", name=f"wq{c}")
                     for c in range(KC_E)]
            wk_sb = [wts.tile([128, 256], DT_BF, tag=f"wk{c}", name=f"wk{c}")
                     for c in range(KC_E)]
            wv_sb = [wts.tile([128, 256], DT_BF, tag=f"wv{c}", name=f"wv{c}")
                     for c in range(KC_E)]
            xq_sb = [xin.tile([128, S], DT_BF, tag=f"xq{c}", name=f"xq{c}")
                     for c in range(KC_E)]
            xk_sb = [xin.tile([128, S], DT_BF, tag=f"xk{c}", name=f"xk{c}")
                     for c in range(KC_E)]
            xv_sb = [xin.tile([128, S], DT_BF, tag=f"xv{c}", name=f"xv{c}")
                     for c in range(KC_E)]
            wo_sb = [wts.tile([128, E], DT_BF, tag=f"wo{c}", name=f"wo{c}")
                     for c in range(2)]

            for c in range(KC_E):
                nc.gpsimd.dma_start(out=wk_sb[c], in_=wk[c * 128:(c + 1) * 128, :])
            for c in range(KC_E):
                nc.gpsimd.dma_start(out=wq_sb[c], in_=wq[c * 128:(c + 1) * 128, :])
            nc.gpsimd.dma_start(out=bk_sb, in_=bk[:, :])
            nc.gpsimd.dma_start(out=bq_sb, in_=bq[:, :])
            nc.gpsimd.dma_start(out=nbias_sb, in_=nbias_in[:, :])
            nc.gpsimd.dma_start(out=sel2, in_=sel2_in[:, :])
            for c in range(KC_E):
                nc.sync.dma_start(out=xk_sb[c], in_=xk[c * 128:(c + 1) * 128, :])
                nc.sync.dma_start(out=xq_sb[c], in_=xq[c * 128:(c + 1) * 128, :])
            for c in range(KC_E):
                nc.gpsimd.dma_start(out=wv_sb[c], in_=wv[c * 128:(c + 1) * 128, :])
            nc.gpsimd.dma_start(out=bv_sb, in_=bv[:, :])
            for c in range(KC_E):
                nc.gpsimd.dma_start(out=xv_sb[c], in_=xv[c * 128:(c + 1) * 128, :])
            for c in range(2):
                nc.gpsimd.dma_start(out=wo_sb[c], in_=wo[c * 128:(c + 1) * 128, :])

            qt = [qkpool.tile([128, S], DT_BF, tag=f"qt{p}", name=f"qt{p}")
                  for p in range(NPAIR)]
            kt = [qkpool.tile([128, S], DT_BF, tag=f"kt{p}", name=f"kt{p}")
                  for p in range(NPAIR)]
            heads_t = [qkpool.tile([128, S], DT_BF, tag=f"ht{p}", name=f"ht{p}")
                       for p in range(NPAIR)]
            # One tile per seq chunk: keeps the PV-read -> v-drain-write
            # dependency trackable at tile granularity.
            v_sb = [vpool.tile([128, HPC, HD + 1], DT_BF, tag=f"v{m}",
                               name=f"v{m}") for m in range(MQ)]

            def proj_norm_unit_ops(dst, w_sb, b_sb, x_sb, mc, n4, pool, ptag,
                                   with_tau):
                """One 512-col projection unit + per-unit norm chain:
                8 accum matmuls + bias matmul -> drain -> square -> sumsq
                matmul -> Ln -> Exp (tau folded into bias) -> bf16 broadcast
                matmul -> in-place normalize."""
                sl = slice(n4 * 512, (n4 + 1) * 512)
                box = {}
                ops = []

                def mk_mm(c):
                    def go():
                        if c == 0:
                            box["pp"] = pool.tile([128, 512], DT_F32,
                                                  tag=ptag, name="pp_t")
                        nc.tensor.matmul(
                            box["pp"],
                            lhsT=w_sb[c][:, mc * 128:(mc + 1) * 128],
                            rhs=x_sb[c][:, sl],
                            start=(c == 0),
                            stop=False,
                        )
                    return go

                for c in range(KC_E):
                    ops.append((220, mk_mm(c)))

                def bias_mm():
                    nc.tensor.matmul(
                        box["pp"],
                        lhsT=b_sb[0:1, mc * 128:(mc + 1) * 128],
                        rhs=ones_row[0:1, 0:512],
                        start=False,
                        stop=True,
                    )
                ops.append((220, bias_mm))

                def drain_sq():
                    nc.vector.tensor_copy(out=dst[:, sl], in_=box["pp"])
                    sq = sqp.tile([128, 512], DT_BF, tag="sq", name="sq_t")
                    nc.vector.tensor_mul(sq, dst[:, sl], dst[:, sl])
                    box["sq"] = sq
                ops.append((0, drain_sq))

                def sumsq():
                    ss = ps_aux.tile([2, 512], DT_F32, tag="aux", name="ss_t")
                    nc.tensor.matmul(ss, lhsT=hsel, rhs=box["sq"],
                                     start=True, stop=True)
                    box["ss"] = ss
                ops.append((220, sumsq))

                def ln_exp():
                    lt2 = wk2.tile([2, 512], DT_F32, tag="lt2", name="lt2_t")
                    nc.scalar.activation(lt2, box["ss"],
                                         mybir.ActivationFunctionType.Ln)
                    rr2 = wk2.tile([2, 512], DT_BF, tag="rr2", name="rr2_t")
                    bias = nbias_sb[:, mc:mc + 1] if with_tau else 0.0
                    nc.scalar.activation(rr2, lt2,
                                         mybir.ActivationFunctionType.Exp,
                                         bias=bias, scale=-0.5)
                    box["rr2"] = rr2
                ops.append((0, ln_exp))

                def bcast():
                    rb = ps_fill.tile([128, 512], DT_F32, tag="fill",
                                      name="rb_t")
                    nc.tensor.matmul(rb, lhsT=sel2, rhs=box["rr2"],
                                     start=True, stop=True)
                    box["rb"] = rb
                ops.append((220, bcast))

                def apply():
                    nc.vector.tensor_mul(dst[:, sl], dst[:, sl], box["rb"])
                ops.append((0, apply))
                return ops

            def v_unit_ops(m):
                """v projection for seq chunk m; drain is 4 contiguous
                per-head copies + 4 one-column memsets (strided/rearranged
                SBUF writes lose RAW tracking)."""
                box = {}
                ops = []

                def mk_mm(c):
                    def go():
                        if c == 0:
                            box["vp"] = ps_fill.tile([128, 256], DT_F32,
                                                     tag="fill", name="vp_t")
                        nc.tensor.matmul(
                            box["vp"],
                            lhsT=xv_sb[c][:, m * 128:(m + 1) * 128],
                            rhs=wv_sb[c],
                            start=(c == 0),
                            stop=False,
                        )
                    return go

                for c in range(KC_E):
                    ops.append((115, mk_mm(c)))

                def bias_mm():
                    nc.tensor.matmul(
                        box["vp"],
                        lhsT=ones_row[0:1, 0:128],
                        rhs=bv_sb[0:1, :],
                        start=False,
                        stop=True,
                    )
                ops.append((115, bias_mm))

                def drain():
                    for h in range(HPC):
                        nc.vector.tensor_copy(
                            out=v_sb[m][:, h, 0:HD],
                            in_=box["vp"][:, h * HD:(h + 1) * HD],
                        )
                        nc.vector.memset(v_sb[m][:, h, HD:HD + 1], 1.0)
                ops.append((0, drain, ("v", m)))
                return ops

            def outproj_unit_ops(m, n2):
                sl_n = slice(n2 * 512, (n2 + 1) * 512)
                box = {}
                ops = []

                def mk_mm(c):
                    def go():
                        if c == 0:
                            box["op"] = ps_fill.tile([128, 512], DT_F32,
                                                     tag="fill", name="op_t")
                        nc.tensor.matmul(
                            box["op"],
                            lhsT=heads_t[c][:, m * 128:(m + 1) * 128],
                            rhs=wo_sb[c][:, sl_n],
                            start=(c == 0), stop=(c == 1),
                        )
                    return go

                ops.append((220, mk_mm(0)))
                ops.append((220, mk_mm(1)))

                def drain():
                    ob = outs.tile([128, 512], DT_BF, tag="ob", name="ob_t")
                    nc.vector.tensor_copy(ob, box["op"])
                    nc.sync.dma_start(
                        out=outp[m * 128:(m + 1) * 128, sl_n], in_=ob
                    )
                ops.append((0, drain))
                return ops

            done_markers = set()

            def _pop_one(fq):
                item = fq.popleft()
                item[1]()
                if len(item) > 2:
                    done_markers.add(item[2])
                return item[0]

            def pop_fillers(fq, budget):
                while fq and budget > 0:
                    budget -= _pop_one(fq)

            def ensure_emitted(fq, marker):
                """Tile deps are created in emission order: anything a
                previously-emitted op reads must already be emitted."""
                if marker in done_markers:
                    return
                if not any(len(it) > 2 and it[2] == marker for it in fq):
                    return
                while fq and marker not in done_markers:
                    _pop_one(fq)

            def attention_pair(p, fq, after_qb=None):
                """Attention for head pair p. Pops filler ops between
                iterations; per-qb softmax drain (bf16 z broadcast + DVE
                fast-recip at base 0) is deferred into the next qb's
                iteration 2 so the exp stream flows across boundaries."""
                def mk_drain(qb, o0, o1):
                    sl_q = slice(qb * 512, (qb + 1) * 512)

                    def emit_drain():
                        for hl, o in ((0, o0), (1, o1)):
                            zcb = work.tile([128, 512], DT_BF, tag="zcb",
                                            name="zcb_t")
                            nc.vector.tensor_copy(zcb[64:65, :], o[64:65, :])
                            zb = ps_aux.tile([64, 512], DT_F32, tag="aux",
                                             name="zb_t")
                            nc.tensor.matmul(
                                zb,
                                lhsT=ones_hi[64:65, 0:64],
                                rhs=zcb[64:65, :],
                                start=True, stop=True,
                            )
                            zbi = work.tile([64, 512], DT_F32, tag="zbi",
                                            name="zbi_t")
                            nc.vector.reciprocal_approx_fast(out=zbi, in_=zb)
                            if hl == 0:
                                nc.vector.tensor_mul(
                                    heads_t[p][0:64, sl_q], o[0:64, :], zbi
                                )
                            else:
                                t2 = work.tile([64, 512], DT_BF, tag="t2",
                                               name="t2_t")
                                nc.vector.tensor_mul(t2, o[0:64, :], zbi)
                                nc.sync.dma_start(
                                    out=heads_t[p][64:128, sl_q], in_=t2
                                )
                        if after_qb is not None:
                            after_qb(qb)
                    return emit_drain

                pending_drain = None
                for qb in range(4):
                    sl_q = slice(qb * 512, (qb + 1) * 512)
                    o0 = ps_acc.tile([128, 512], DT_F32, tag="oacc", name="o0_t")
                    o1 = ps_acc.tile([128, 512], DT_F32, tag="oacc", name="o1_t")
                    for kc in range(MQ):
                        sc = ps_sc.tile([128, 1024], DT_F32, tag="sc", name="sc_t")
                        nc.tensor.matmul(
                            sc[:, 0:512],
                            lhsT=kt[p][0:64, kc * 128:(kc + 1) * 128],
                            rhs=qt[p][0:64, sl_q],
                            start=True, stop=True,
                        )
                        nc.tensor.matmul(
                            sc[:, 512:1024],
                            lhsT=kt[p][64:128, kc * 128:(kc + 1) * 128],
                            rhs=qt[p][64:128, sl_q],
                            start=True, stop=True,
                        )
                        ex = work.tile([128, 1024], DT_BF, tag="exp", name="ex_t")
                        nc.scalar.activation(
                            ex, sc, mybir.ActivationFunctionType.Exp
                        )
                        if kc == 2 and pending_drain is not None:
                            pending_drain()
                            pending_drain = None
                        ensure_emitted(fq, ("v", kc))
                        nc.tensor.matmul(
                            o0[0:65, :],
                            lhsT=v_sb[kc][:, 2 * p, :],
                            rhs=ex[:, 0:512],
                            start=(kc == 0), stop=(kc == MQ - 1),
                        )
                        nc.tensor.matmul(
                            o1[0:65, :],
                            lhsT=v_sb[kc][:, 2 * p + 1, :],
                            rhs=ex[:, 512:1024],
                            start=(kc == 0), stop=(kc == MQ - 1),
                        )
                        pop_fillers(fq, ITER_FILL_NS)
                    pending_drain = mk_drain(qb, o0, o1)
                pending_drain()

            # ---- lead-in: k0 then q0, per-unit norm chains -----------------
            for n4 in range(4):
                for item in proj_norm_unit_ops(kt[0], wk_sb, bk_sb, xk_sb,
                                               0, n4, ps_sc, "sc", True):
                    item[1]()
            for n4 in range(4):
                for item in proj_norm_unit_ops(qt[0], wq_sb, bq_sb, xq_sb,
                                               0, n4, ps_sc, "sc", False):
                    item[1]()
            for m in range(3):
                for item in v_unit_ops(m):
                    item[1]()

            # ---- pair-0 attention + fillers --------------------------------
            fq = deque()
            for m in range(3, MQ):
                fq.extend(v_unit_ops(m))
            for n4 in range(4):
                fq.extend(proj_norm_unit_ops(kt[1], wk_sb, bk_sb, xk_sb,
                                             1, n4, ps_fill, "fill", True))
            for n4 in range(4):
                fq.extend(proj_norm_unit_ops(qt[1], wq_sb, bq_sb, xq_sb,
                                             1, n4, ps_fill, "fill", False))
            attention_pair(0, fq)

            # ---- pair-1 attention + out-projection fillers -----------------
            def after_qb(qb):
                for m in range(qb * 4, qb * 4 + 4):
                    for n2 in range(2):
                        fq.extend(outproj_unit_ops(m, n2))

            attention_pair(1, fq, after_qb=after_qb)
            while fq:
                _pop_one(fq)

    nc.compile()
    return nc


_CACHE = {}


def _get_program():
    if "nc" not in _CACHE:
        _CACHE["nc"] = build_program()
    return _CACHE["nc"]


def make_in_maps(query, key, value, in_proj_weight, in_proj_bias,
                 out_proj_weight, out_proj_bias, tau):
    query = np.asarray(query, np.float32)
    key = np.asarray(key, np.float32)
    value = np.asarray(value, np.float32)
    W = np.asarray(in_proj_weight, np.float32)
    bias = np.asarray(in_proj_bias, np.float32)
    Wo = np.asarray(out_proj_weight, np.float32)
    tau_c = np.maximum(np.asarray(tau, np.float32).reshape(H), TAU_MIN)

    # Transposed activations per batch: (E, S) bf16
    xT = {}
    for b in range(B):
        xT["q", b] = np.ascontiguousarray(query[:, b, :].T).astype(BF16)
        xT["k", b] = np.ascontiguousarray(key[:, b, :].T).astype(BF16)
        xT["v", b] = np.ascontiguousarray(value[:, b, :].T).astype(BF16)

    sel2_host = np.zeros((2, 128), np.float32)
    sel2_host[0, 0:64] = 1.0
    sel2_host[1, 64:128] = 1.0
    sel2_host = sel2_host.astype(BF16)

    in_maps = []
    for c in range(NCORES):
        b = c // 4
        h0 = HPC * (c % 4)
        rows = slice(h0 * HD, (h0 + HPC) * HD)
        rows_k = slice(E + h0 * HD, E + (h0 + HPC) * HD)
        rows_v = slice(2 * E + h0 * HD, 2 * E + (h0 + HPC) * HD)
        # per-pair -ln(tau) fold for the k-side norms: [head_lr, pair]
        nbias = np.zeros((2, 2), np.float32)
        for p in range(NPAIR):
            nbias[0, p] = -np.log(tau_c[h0 + 2 * p])
            nbias[1, p] = -np.log(tau_c[h0 + 2 * p + 1])
        in_maps.append({
            "xq_t": xT["q", b],
            "xk_t": xT["k", b],
            "xv_t": xT["v", b],
            "wq_t": np.ascontiguousarray(W[rows, :].T).astype(BF16),
            "wk_t": np.ascontiguousarray(W[rows_k, :].T).astype(BF16),
            "wv_t": np.ascontiguousarray(W[rows_v, :].T).astype(BF16),
            "b_q": bias[rows].reshape(1, 256).astype(BF16),
            "b_k": bias[rows_k].reshape(1, 256).astype(BF16),
            "b_v": bias[rows_v].reshape(1, 256).astype(BF16),
            "wo_t": np.ascontiguousarray(Wo[:, rows].T).astype(BF16),
            "nbias": nbias,
            "sel2": sel2_host,
        })
    return in_maps


def assemble_out(results, out_proj_bias):
    bo = np.asarray(out_proj_bias, np.float32)
    out = np.zeros((S, B, E), np.float32)
    for c in range(NCORES):
        out[:, c // 4, :] += np.asarray(results[c]["out_p"], np.float32)
    out += bo[None, None, :]
    return out


def kernel(query, key, value, in_proj_weight, in_proj_bias,
           out_proj_weight, out_proj_bias, tau):
    from concourse.bass_utils import run_bass_kernel_spmd
    nc = _get_program()
    in_maps = make_in_maps(query, key, value, in_proj_weight, in_proj_bias,
                           out_proj_weight, out_proj_bias, tau)
    res = run_bass_kernel_spmd(nc, in_maps, core_ids=list(range(NCORES)))
    return assemble_out(res.results, out_proj_bias)


if __name__ == "__main__":
    import reference

    inputs = {k: np.asarray(v) for k, v in reference.setup_inputs().items()}
    out = kernel(**inputs)
    print("out shape", out.shape, out.dtype)


# revision 45
# speedup vs baseline: 1.2660x; 1.0023x over previous
"""Cosine multihead attention on 8 Trainium2 NeuronCores.

Sharding: batch*heads across cores. Core c handles batch b = c // 4 and the
4 heads [4*(c%4), 4*(c%4)+4). Each core computes its heads' q/k/v projections
(tensor-parallel slices of in_proj), full attention for its (B,H) slice, and a
partial bf16 out-projection (rank-256 contribution). The host sums the 4
partials per batch and adds out_proj_bias.

Schedule: steady state is ACT(exp)-bound (one [128,1024] Exp per (qb,kc)
attention iteration, ~1.15us), with ~0.5us/iter of PE slack. v projection,
pair-1 q/k projections+norms and the out-projection are interleaved into that
slack via a filler deque popped between attention iterations. Tile creates
dependencies in EMISSION order, so any filler writing data that attention
reads must be emitted before the reading op: v units carry markers that
ensure_emitted() forces out before the PV that consumes them.

Norms use one ACT table set (natural_log_exp_and_others, manually preloaded +
drained once): 1/(||x||*tau) = Exp(-0.5*Ln(sumsq) - ln tau). No sqrt set, so
no table thrash against the attention Exp stream; norm broadcasts are bf16
matmuls. Softmax denominators come free from a ones-column in v (M=65 PV);
per-qb drains are deferred into the next qb's iteration 2 so the exp stream
flows across boundaries.

Device layout:
- q,k projected transposed (head_dim on partitions, seq on free) so QK^T
  needs no on-chip transpose; v natural so it is the PV stationary operand.
- QK^T runs 2 heads concurrently via PE row tiling (K=64 at bases 0 and 64).
- All bf16 matmuls with fp32 PSUM accumulation; softmax math in fp32.
"""

import sys
from collections import deque

if "/opt/trn_rl_repo" not in sys.path:
    sys.path.insert(0, "/opt/trn_rl_repo")

import numpy as np
import ml_dtypes

import concourse.bass as bass
import concourse.tile as tile
from concourse import bacc, mybir

S, B, E, H = 2048, 2, 1024, 16
HD = E // H            # 64
HPC = 4                # heads per core
NCORES = 8
TAU_MIN = 0.01

BF16 = ml_dtypes.bfloat16
DT_BF = mybir.dt.bfloat16
DT_F32 = mybir.dt.float32

KC_E = E // 128        # 8 contraction chunks for projections
MQ = S // 128          # 16 seq chunks of 128
NPAIR = HPC // 2       # 2 head pairs per core

ACT_SET_LN_EXP = 6     # natural_log_exp_and_others in act_info.json
ITER_FILL_NS = 560     # PE-time filler budget per attention iteration


def build_program():
    nc = bacc.Bacc(None)

    xq = nc.dram_tensor("xq_t", [E, S], DT_BF, kind="ExternalInput")
    xk = nc.dram_tensor("xk_t", [E, S], DT_BF, kind="ExternalInput")
    xv = nc.dram_tensor("xv_t", [E, S], DT_BF, kind="ExternalInput")
    wq = nc.dram_tensor("wq_t", [E, 256], DT_BF, kind="ExternalInput")
    wk = nc.dram_tensor("wk_t", [E, 256], DT_BF, kind="ExternalInput")
    wv = nc.dram_tensor("wv_t", [E, 256], DT_BF, kind="ExternalInput")
    bq = nc.dram_tensor("b_q", [1, 256], DT_BF, kind="ExternalInput")
    bk = nc.dram_tensor("b_k", [1, 256], DT_BF, kind="ExternalInput")
    bv = nc.dram_tensor("b_v", [1, 256], DT_BF, kind="ExternalInput")
    wo = nc.dram_tensor("wo_t", [256, E], DT_BF, kind="ExternalInput")
    nbias_in = nc.dram_tensor("nbias", [2, 2], DT_F32, kind="ExternalInput")
    sel2_in = nc.dram_tensor("sel2", [2, 128], DT_BF, kind="ExternalInput")
    outp = nc.dram_tensor("out_p", [S, E], DT_BF, kind="ExternalOutput")

    with tile.TileContext(nc) as tc:
        with (
            tc.tile_pool(name="consts", bufs=1) as consts,
            tc.tile_pool(name="xin", bufs=1) as xin,
            tc.tile_pool(name="wts", bufs=1) as wts,
            tc.tile_pool(name="qk", bufs=1) as qkpool,
            tc.tile_pool(name="vsb", bufs=1) as vpool,
            tc.tile_pool(name="work", bufs=3) as work,
            tc.tile_pool(name="wk2", bufs=2) as wk2,
            tc.tile_pool(name="sqp", bufs=2) as sqp,
            tc.tile_pool(name="outs", bufs=3) as outs,
            tc.tile_pool(name="ps_sc", bufs=2, space="PSUM") as ps_sc,
            tc.tile_pool(name="ps_acc", bufs=2, space="PSUM") as ps_acc,
            tc.tile_pool(name="ps_aux", bufs=1, space="PSUM") as ps_aux,
            tc.tile_pool(name="ps_fill", bufs=1, space="PSUM") as ps_fill,
        ):
            # ---- ACT table preload: combined ln+exp set, loaded once.
            # The DRAIN is required: the table DMA is async and the first
            # ACTIVATE would race it on the first execution.
            nc.scalar.add_instruction(
                mybir.InstLoadActFuncSet(
                    name=nc.get_next_instruction_name(),
                    act_func_set_id=ACT_SET_LN_EXP,
                    ins=[],
                    outs=[],
                )
            )
            drain = mybir.InstDrain(
                name=nc.get_next_instruction_name(),
                ins=[],
                outs=[],
                bass_is_fusable=False,
            )
            drain.engine = nc.scalar.engine
            nc.scalar.add_instruction(drain)

            # ---- constants -------------------------------------------------
            ones_row = consts.tile([1, 512], DT_BF, tag="ones_row")
            nc.vector.memset(ones_row, 1.0)
            ones_hi = consts.tile([128, 64], DT_BF, tag="ones_hi")
            nc.vector.memset(ones_hi, 1.0)
            hsel = consts.tile([128, 2], DT_BF, tag="hsel")
            nc.vector.memset(hsel, 0.0)
            nc.vector.memset(hsel[0:64, 0:1], 1.0)
            nc.vector.memset(hsel[64:128, 1:2], 1.0)
            # table-warm dummy (nothing consumes it)
            warm = consts.tile([1, 64], DT_F32, tag="warm")
            nc.vector.memset(warm, 1.0)
            nc.scalar.activation(warm, warm, mybir.ActivationFunctionType.Exp)

            sel2 = consts.tile([2, 128], DT_BF, tag="sel2")
            nbias_sb = consts.tile([2, 2], DT_F32, tag="nbias")
            bq_sb = consts.tile([1, 256], DT_BF, tag="bq")
            bk_sb = consts.tile([1, 256], DT_BF, tag="bk")
            bv_sb = consts.tile([1, 256], DT_BF, tag="bv")

            # ---- DMA plan --------------------------------------------------
            # gpsimd(SWDGE): weights/consts then xv then wo
            # sync(HWDGE):   xk and xq (k first per chunk)
            # Per-chunk input tiles: Tile RAW deps are tile-granular, so the
            # projections become ready chunk by chunk as DMAs land.
            wq_sb = [wts.tile([128, 256], DT_BF, tag=f"wq{c}", name=f"wq{c}")
                     for c in range(KC_E)]
            wk_sb = [wts.tile([128, 256], DT_BF, tag=f"wk{c}", name=f"wk{c}")
                     for c in range(KC_E)]
            wv_sb = [wts.tile([128, 256], DT_BF, tag=f"wv{c}", name=f"wv{c}")
                     for c in range(KC_E)]
            xq_sb = [xin.tile([128, S], DT_BF, tag=f"xq{c}", name=f"xq{c}")
                     for c in range(KC_E)]
            xk_sb = [xin.tile([128, S], DT_BF, tag=f"xk{c}", name=f"xk{c}")
                     for c in range(KC_E)]
            xv_sb = [xin.tile([128, S], DT_BF, tag=f"xv{c}", name=f"xv{c}")
                     for c in range(KC_E)]
            wo_sb = [wts.tile([128, E], DT_BF, tag=f"wo{c}", name=f"wo{c}")
                     for c in range(2)]

            for c in range(KC_E):
                nc.gpsimd.dma_start(out=wk_sb[c], in_=wk[c * 128:(c + 1) * 128, :])
            for c in range(KC_E):
                nc.gpsimd.dma_start(out=wq_sb[c], in_=wq[c * 128:(c + 1) * 128, :])
            nc.gpsimd.dma_start(out=bk_sb, in_=bk[:, :])
            nc.gpsimd.dma_start(out=bq_sb, in_=bq[:, :])
            nc.gpsimd.dma_start(out=nbias_sb, in_=nbias_in[:, :])
            nc.gpsimd.dma_start(out=sel2, in_=sel2_in[:, :])
            for c in range(KC_E):
                nc.sync.dma_start(out=xk_sb[c], in_=xk[c * 128:(c + 1) * 128, :])
                nc.scalar.dma_start(out=xq_sb[c], in_=xq[c * 128:(c + 1) * 128, :])
            for c in range(KC_E):
                nc.gpsimd.dma_start(out=wv_sb[c], in_=wv[c * 128:(c + 1) * 128, :])
            nc.gpsimd.dma_start(out=bv_sb, in_=bv[:, :])
            for c in range(KC_E):
                nc.gpsimd.dma_start(out=xv_sb[c], in_=xv[c * 128:(c + 1) * 128, :])
            for c in range(2):
                nc.gpsimd.dma_start(out=wo_sb[c], in_=wo[c * 128:(c + 1) * 128, :])

            qt = [qkpool.tile([128, S], DT_BF, tag=f"qt{p}", name=f"qt{p}")
                  for p in range(NPAIR)]
            kt = [qkpool.tile([128, S], DT_BF, tag=f"kt{p}", name=f"kt{p}")
                  for p in range(NPAIR)]
            heads_t = [qkpool.tile([128, S], DT_BF, tag=f"ht{p}", name=f"ht{p}")
                       for p in range(NPAIR)]
            # One tile per seq chunk: keeps the PV-read -> v-drain-write
            # dependency trackable at tile granularity.
            v_sb = [vpool.tile([128, HPC, HD + 1], DT_BF, tag=f"v{m}",
                               name=f"v{m}") for m in range(MQ)]

            def proj_norm_unit_ops(dst, w_sb, b_sb, x_sb, mc, n4, pool, ptag,
                                   with_tau):
                """One 512-col projection unit + per-unit norm chain:
                8 accum matmuls + bias matmul -> drain -> square -> sumsq
                matmul -> Ln -> Exp (tau folded into bias) -> bf16 broadcast
                matmul -> in-place normalize."""
                sl = slice(n4 * 512, (n4 + 1) * 512)
                box = {}
                ops = []

                def mk_mm(c):
                    def go():
                        if c == 0:
                            box["pp"] = pool.tile([128, 512], DT_F32,
                                                  tag=ptag, name="pp_t")
                        nc.tensor.matmul(
                            box["pp"],
                            lhsT=w_sb[c][:, mc * 128:(mc + 1) * 128],
                            rhs=x_sb[c][:, sl],
                            start=(c == 0),
                            stop=False,
                        )
                    return go

                for c in range(KC_E):
                    ops.append((220, mk_mm(c)))

                def bias_mm():
                    nc.tensor.matmul(
                        box["pp"],
                        lhsT=b_sb[0:1, mc * 128:(mc + 1) * 128],
                        rhs=ones_row[0:1, 0:512],
                        start=False,
                        stop=True,
                    )
                ops.append((220, bias_mm))

                def drain_sq():
                    nc.vector.tensor_copy(out=dst[:, sl], in_=box["pp"])
                    sq = sqp.tile([128, 512], DT_BF, tag="sq", name="sq_t")
                    nc.vector.tensor_mul(sq, dst[:, sl], dst[:, sl])
                    box["sq"] = sq
                ops.append((0, drain_sq))

                def sumsq():
                    ss = ps_aux.tile([2, 512], DT_F32, tag="aux", name="ss_t")
                    nc.tensor.matmul(ss, lhsT=hsel, rhs=box["sq"],
                                     start=True, stop=True)
                    box["ss"] = ss
                ops.append((220, sumsq))

                def ln_exp():
                    lt2 = wk2.tile([2, 512], DT_F32, tag="lt2", name="lt2_t")
                    nc.scalar.activation(lt2, box["ss"],
                                         mybir.ActivationFunctionType.Ln)
                    rr2 = wk2.tile([2, 512], DT_BF, tag="rr2", name="rr2_t")
                    bias = nbias_sb[:, mc:mc + 1] if with_tau else 0.0
                    nc.scalar.activation(rr2, lt2,
                                         mybir.ActivationFunctionType.Exp,
                                         bias=bias, scale=-0.5)
                    box["rr2"] = rr2
                ops.append((0, ln_exp))

                def bcast():
                    rb = ps_fill.tile([128, 512], DT_F32, tag="fill",
                                      name="rb_t")
                    nc.tensor.matmul(rb, lhsT=sel2, rhs=box["rr2"],
                                     start=True, stop=True)
                    box["rb"] = rb
                ops.append((220, bcast))

                def apply():
                    nc.vector.tensor_mul(dst[:, sl], dst[:, sl], box["rb"])
                ops.append((0, apply))
                return ops

            def v_unit_ops(m):
                """v projection for seq chunk m; drain is 4 contiguous
                per-head copies + 4 one-column memsets (strided/rearranged
                SBUF writes lose RAW tracking)."""
                box = {}
                ops = []

                def mk_mm(c):
                    def go():
                        if c == 0:
                            box["vp"] = ps_fill.tile([128, 256], DT_F32,
                                                     tag="fill", name="vp_t")
                        nc.tensor.matmul(
                            box["vp"],
                            lhsT=xv_sb[c][:, m * 128:(m + 1) * 128],
                            rhs=wv_sb[c],
                            start=(c == 0),
                            stop=False,
                        )
                    return go

                for c in range(KC_E):
                    ops.append((115, mk_mm(c)))

                def bias_mm():
                    nc.tensor.matmul(
                        box["vp"],
                        lhsT=ones_row[0:1, 0:128],
                        rhs=bv_sb[0:1, :],
                        start=False,
                        stop=True,
                    )
                ops.append((115, bias_mm))

                def drain():
                    for h in range(HPC):
                        nc.vector.tensor_copy(
                            out=v_sb[m][:, h, 0:HD],
                            in_=box["vp"][:, h * HD:(h + 1) * HD],
                        )
                        nc.vector.memset(v_sb[m][:, h, HD:HD + 1], 1.0)
                ops.append((0, drain, ("v", m)))
                return ops

            def outproj_unit_ops(m, n2):
                sl_n = slice(n2 * 512, (n2 + 1) * 512)
                box = {}
                ops = []

                def mk_mm(c):
                    def go():
                        if c == 0:
                            box["op"] = ps_fill.tile([128, 512], DT_F32,
                                                     tag="fill", name="op_t")
                        nc.tensor.matmul(
                            box["op"],
                            lhsT=heads_t[c][:, m * 128:(m + 1) * 128],
                            rhs=wo_sb[c][:, sl_n],
                            start=(c == 0), stop=(c == 1),
                        )
                    return go

                ops.append((220, mk_mm(0)))
                ops.append((220, mk_mm(1)))

                def drain():
                    ob = outs.tile([128, 512], DT_BF, tag="ob", name="ob_t")
                    nc.vector.tensor_copy(ob, box["op"])
                    nc.sync.dma_start(
                        out=outp[m * 128:(m + 1) * 128, sl_n], in_=ob
                    )
                ops.append((0, drain))
                return ops

            done_markers = set()

            def _pop_one(fq):
                item = fq.popleft()
                item[1]()
                if len(item) > 2:
                    done_markers.add(item[2])
                return item[0]

            def pop_fillers(fq, budget):
                while fq and budget > 0:
                    budget -= _pop_one(fq)

            def ensure_emitted(fq, marker):
                """Tile deps are created in emission order: anything a
                previously-emitted op reads must already be emitted."""
                if marker in done_markers:
                    return
                if not any(len(it) > 2 and it[2] == marker for it in fq):
                    return
                while fq and marker not in done_markers:
                    _pop_one(fq)

            def attention_pair(p, fq, after_qb=None):
                """Attention for head pair p. Pops filler ops between
                iterations; per-qb softmax drain (bf16 z broadcast + DVE
                fast-recip at base 0) is deferred into the next qb's
                iteration 2 so the exp stream flows across boundaries."""
                def mk_drain(qb, o0, o1):
                    sl_q = slice(qb * 512, (qb + 1) * 512)

                    def emit_drain():
                        for hl, o in ((0, o0), (1, o1)):
                            zcb = work.tile([128, 512], DT_BF, tag="zcb",
                                            name="zcb_t")
                            nc.vector.tensor_copy(zcb[64:65, :], o[64:65, :])
                            zb = ps_aux.tile([64, 512], DT_F32, tag="aux",
                                             name="zb_t")
                            nc.tensor.matmul(
                                zb,
                                lhsT=ones_hi[64:65, 0:64],
                                rhs=zcb[64:65, :],
                                start=True, stop=True,
                            )
                            zbi = work.tile([64, 512], DT_F32, tag="zbi",
                                            name="zbi_t")
                            nc.vector.reciprocal_approx_fast(out=zbi, in_=zb)
                            if hl == 0:
                                nc.vector.tensor_mul(
                                    heads_t[p][0:64, sl_q], o[0:64, :], zbi
                                )
                            else:
                                t2 = work.tile([64, 512], DT_BF, tag="t2",
                                               name="t2_t")
                                nc.vector.tensor_mul(t2, o[0:64, :], zbi)
                                nc.sync.dma_start(
                                    out=heads_t[p][64:128, sl_q], in_=t2
                                )
                        if after_qb is not None:
                            after_qb(qb)
                    return emit_drain

                pending_drain = None
                for qb in range(4):
                    sl_q = slice(qb * 512, (qb + 1) * 512)
                    o0 = ps_acc.tile([128, 512], DT_F32, tag="oacc", name="o0_t")
                    o1 = ps_acc.tile([128, 512], DT_F32, tag="oacc", name="o1_t")
                    for kc in range(MQ):
                        sc = ps_sc.tile([128, 1024], DT_F32, tag="sc", name="sc_t")
                        nc.tensor.matmul(
                            sc[:, 0:512],
                            lhsT=kt[p][0:64, kc * 128:(kc + 1) * 128],
                            rhs=qt[p][0:64, sl_q],
                            start=True, stop=True,
                        )
                        nc.tensor.matmul(
                            sc[:, 512:1024],
                            lhsT=kt[p][64:128, kc * 128:(kc + 1) * 128],
                            rhs=qt[p][64:128, sl_q],
                            start=True, stop=True,
                        )
                        ex = work.tile([128, 1024], DT_BF, tag="exp", name="ex_t")
                        nc.scalar.activation(
                            ex, sc, mybir.ActivationFunctionType.Exp
                        )
                        if kc == 2 and pending_drain is not None:
                            pending_drain()
                            pending_drain = None
                        ensure_emitted(fq, ("v", kc))
                        nc.tensor.matmul(
                            o0[0:65, :],
                            lhsT=v_sb[kc][:, 2 * p, :],
                            rhs=ex[:, 0:512],
                            start=(kc == 0), stop=(kc == MQ - 1),
                        )
                        nc.tensor.matmul(
                            o1[0:65, :],
                            lhsT=v_sb[kc][:, 2 * p + 1, :],
                            rhs=ex[:, 512:1024],
                            start=(kc == 0), stop=(kc == MQ - 1),
                        )
                        pop_fillers(fq, ITER_FILL_NS)
                    pending_drain = mk_drain(qb, o0, o1)
                pending_drain()

            # ---- lead-in: k0 then q0, per-unit norm chains -----------------
            for n4 in range(4):
                for item in proj_norm_unit_ops(kt[0], wk_sb, bk_sb, xk_sb,
                                               0, n4, ps_sc, "sc", True):
                    item[1]()
            for n4 in range(4):
                for item in proj_norm_unit_ops(qt[0], wq_sb, bq_sb, xq_sb,
                                               0, n4, ps_sc, "sc", False):
                    item[1]()
            for m in range(3):
                for item in v_unit_ops(m):
                    item[1]()

            # ---- pair-0 attention + fillers --------------------------------
            fq = deque()
            for m in range(3, MQ):
                fq.extend(v_unit_ops(m))
            for n4 in range(4):
                fq.extend(proj_norm_unit_ops(kt[1], wk_sb, bk_sb, xk_sb,
                                             1, n4, ps_fill, "fill", True))
            for n4 in range(4):
                fq.extend(proj_norm_unit_ops(qt[1], wq_sb, bq_sb, xq_sb,
                                             1, n4, ps_fill, "fill", False))
            attention_pair(0, fq)

            # ---- pair-1 attention + out-projection fillers -----------------
            def after_qb(qb):
                for m in range(qb * 4, qb * 4 + 4):
                    for n2 in range(2):
                        fq.extend(outproj_unit_ops(m, n2))

            attention_pair(1, fq, after_qb=after_qb)
            while fq:
                _pop_one(fq)

    nc.compile()
    return nc


_CACHE = {}


def _get_program():
    if "nc" not in _CACHE:
        _CACHE["nc"] = build_program()
    return _CACHE["nc"]


def make_in_maps(query, key, value, in_proj_weight, in_proj_bias,
                 out_proj_weight, out_proj_bias, tau):
    query = np.asarray(query, np.float32)
    key = np.asarray(key, np.float32)
    value = np.asarray(value, np.float32)
    W = np.asarray(in_proj_weight, np.float32)
    bias = np.asarray(in_proj_bias, np.float32)
    Wo = np.asarray(out_proj_weight, np.float32)
    tau_c = np.maximum(np.asarray(tau, np.float32).reshape(H), TAU_MIN)

    # Transposed activations per batch: (E, S) bf16
    xT = {}
    for b in range(B):
        xT["q", b] = np.ascontiguousarray(query[:, b, :].T).astype(BF16)
        xT["k", b] = np.ascontiguousarray(key[:, b, :].T).astype(BF16)
        xT["v", b] = np.ascontiguousarray(value[:, b, :].T).astype(BF16)

    sel2_host = np.zeros((2, 128), np.float32)
    sel2_host[0, 0:64] = 1.0
    sel2_host[1, 64:128] = 1.0
    sel2_host = sel2_host.astype(BF16)

    in_maps = []
    for c in range(NCORES):
        b = c // 4
        h0 = HPC * (c % 4)
        rows = slice(h0 * HD, (h0 + HPC) * HD)
        rows_k = slice(E + h0 * HD, E + (h0 + HPC) * HD)
        rows_v = slice(2 * E + h0 * HD, 2 * E + (h0 + HPC) * HD)
        # per-pair -ln(tau) fold for the k-side norms: [head_lr, pair]
        nbias = np.zeros((2, 2), np.float32)
        for p in range(NPAIR):
            nbias[0, p] = -np.log(tau_c[h0 + 2 * p])
            nbias[1, p] = -np.log(tau_c[h0 + 2 * p + 1])
        in_maps.append({
            "xq_t": xT["q", b],
            "xk_t": xT["k", b],
            "xv_t": xT["v", b],
            "wq_t": np.ascontiguousarray(W[rows, :].T).astype(BF16),
            "wk_t": np.ascontiguousarray(W[rows_k, :].T).astype(BF16),
            "wv_t": np.ascontiguousarray(W[rows_v, :].T).astype(BF16),
            "b_q": bias[rows].reshape(1, 256).astype(BF16),
            "b_k": bias[rows_k].reshape(1, 256).astype(BF16),
            "b_v": bias[rows_v].reshape(1, 256).astype(BF16),
            "wo_t": np.ascontiguousarray(Wo[:, rows].T).astype(BF16),
            "nbias": nbias,
            "sel2": sel2_host,
        })
    return in_maps


def assemble_out(results, out_proj_bias):
    bo = np.asarray(out_proj_bias, np.float32)
    out = np.zeros((S, B, E), np.float32)
    for c in range(NCORES):
        out[:, c // 4, :] += np.asarray(results[c]["out_p"], np.float32)
    out += bo[None, None, :]
    return out


def kernel(query, key, value, in_proj_weight, in_proj_bias,
           out_proj_weight, out_proj_bias, tau):
    from concourse.bass_utils import run_bass_kernel_spmd
    nc = _get_program()
    in_maps = make_in_maps(query, key, value, in_proj_weight, in_proj_bias,
                           out_proj_weight, out_proj_bias, tau)
    res = run_bass_kernel_spmd(nc, in_maps, core_ids=list(range(NCORES)))
    return assemble_out(res.results, out_proj_bias)


if __name__ == "__main__":
    import reference

    inputs = {k: np.asarray(v) for k, v in reference.setup_inputs().items()}
    out = kernel(**inputs)
    print("out shape", out.shape, out.dtype)
